# revision 16
# baseline (speedup 1.0000x reference)
"""GQA attention prefill kernel for Trainium2 (Bass/Tile), 8-way tensor
parallel over heads.

Problem (hardcoded): B=1, S=2048, HID=4096, NH=32, KVH=8, D=128, causal
prefill with per-head RMSNorm on q/k and RoPE, positions = arange(S).

Sharding: core c owns kv-head c and q-heads 4c..4c+3. wq/wo sharded on the
head dim, wk/wv on the kv-head dim; x, rope tables replicated. Each core
computes its 4 heads' contribution through wo; the host sums the 8 partial
outputs.

v3 design:
- Full bf16 datapath; matmuls accumulate in f32 PSUM. bf16 moving operands
  run at the same 1 cycle/row PE rate as f32r but halve DMA/SBUF traffic
  and double DVE throughput.
- All weights resident in SBUF, loaded via a handful of large batched DMAs
  ordered so chunk-0 projections stream behind the transfers (v2 lost
  ~80us to 100+ serialized DMA triggers before the first x tile).
- Software pipelining: chunk sc's PSUM drains + rope/rms-norm are emitted
  after chunk sc+1's projection matmuls, so DVE/Act work hides under PE.
- PSUM layout: 2-bank [P,2,SC] "dual" tiles (K+V proj pair, Q-head pairs,
  score-tile pairs) + 1-bank "single" tiles (softmax accumulators, out-proj
  groups). Exp is evaluated once per score-tile PAIR (halves Act-engine
  instruction count; the Act engine gated attention in v2).
- Reciprocals via the ~5x faster reciprocal_approx_fast custom DVE op.
- All output-projection PSUM drains on the Vector engine; one y DMA per
  128-row stripe.
"""

import numpy as np

import concourse.bass as bass
import concourse.mybir as mybir
import concourse.tile as tile
from concourse import bacc

P = 128
S = 2048
HID = 4096
D = 128
G = 4            # q heads per core
NHT = HID // P   # 32 h-tiles (contraction)
SC = 512         # seq chunk
NSC = S // SC    # 4
NKT = S // P     # 16 k-tiles
EPS = 1e-6
N_CORES = 8

F32 = mybir.dt.float32
BF16 = mybir.dt.bfloat16

Sqrt = mybir.ActivationFunctionType.Sqrt
Exp = mybir.ActivationFunctionType.Exp


def build_program():
    nc = bacc.Bacc("TRN2", target_bir_lowering=False, debug=False)

    xT = nc.dram_tensor("xT", [HID, S], BF16, kind="ExternalInput").ap()
    wqT = nc.dram_tensor("wqT", [HID, G * P], BF16, kind="ExternalInput").ap()
    wkT = nc.dram_tensor("wkT", [HID, P], BF16, kind="ExternalInput").ap()
    wvT = nc.dram_tensor("wvT", [HID, P], BF16, kind="ExternalInput").ap()
    woT = nc.dram_tensor("woT", [G * P, HID], BF16, kind="ExternalInput").ap()
    cosq = nc.dram_tensor("cosq", [D, S], BF16, kind="ExternalInput").ap()
    sinq = nc.dram_tensor("sinq", [D, S], BF16, kind="ExternalInput").ap()
    cosk = nc.dram_tensor("cosk", [D, S], BF16, kind="ExternalInput").ap()
    sink = nc.dram_tensor("sink", [D, S], BF16, kind="ExternalInput").ap()
    y = nc.dram_tensor("y", [S, HID], BF16, kind="ExternalOutput").ap()

    with tile.TileContext(nc) as tc:
        with (
            tc.tile_pool(name="const", bufs=1) as const,
            tc.tile_pool(name="xw", bufs=3) as xw,
            tc.tile_pool(name="scr", bufs=2) as scr,
            tc.tile_pool(name="ptp", bufs=3) as ptp,
            tc.tile_pool(name="otp", bufs=5) as otp,
            tc.tile_pool(name="ysp", bufs=1) as ysp,
            tc.tile_pool(name="ps", bufs=1, space="PSUM") as ps,
        ):
            # ---- resident weights: batched DMAs, interleaved with the
            # first x blocks so chunk-0 projections stream right behind ----
            wk_sb = const.tile([P, NHT, P], BF16)
            wv_sb = const.tile([P, NHT, P], BF16)
            wq_sb = const.tile([P, NHT, G * P], BF16)
            XB = 4   # ht-tiles per x DMA block
            WQB = 8  # ht-tiles per wq DMA block
            nc.sync.dma_start(
                wk_sb, wkT.rearrange("(ht p) c -> p ht c", p=P))
            nc.sync.dma_start(
                wv_sb, wvT.rearrange("(ht p) c -> p ht c", p=P))
            x0_tiles = []
            for b in range(NHT // WQB):
                xt = xw.tile([P, XB, SC], BF16, tag="xt", name=f"xt0_e{b}")
                nc.sync.dma_start(
                    xt, xT[b * XB * P:(b + 1) * XB * P, 0:SC]
                    .rearrange("(c p) s -> p c s", p=P))
                x0_tiles.append(xt)
                nc.sync.dma_start(
                    wq_sb[:, b * WQB:(b + 1) * WQB, :],
                    wqT[b * WQB * P:(b + 1) * WQB * P, :].rearrange(
                        "(ht p) c -> p ht c", p=P))

            # ---- constants ----
            f32tmp = const.tile([P, SC], F32)
            identity = const.tile([P, P], BF16)
            nc.gpsimd.memset(f32tmp[:, 0:P], 0.0)
            nc.gpsimd.affine_select(
                f32tmp[:, 0:P], f32tmp[:, 0:P],
                compare_op=mybir.AluOpType.not_equal, fill=1.0,
                base=0, pattern=[[-1, P]], channel_multiplier=1,
            )
            nc.vector.tensor_copy(identity, f32tmp[:, 0:P])
            ones_bf = const.tile([P, P], BF16)
            nc.gpsimd.memset(f32tmp[:, 0:P], 1.0)
            nc.vector.tensor_copy(ones_bf, f32tmp[:, 0:P])
            ones_r = const.tile([P, P], mybir.dt.float32r)
            nc.vector.tensor_copy(ones_r, f32tmp[:, 0:P])
            # causal masks for the 4 diagonal k-tiles of a q chunk:
            # keep (1.0) where q_local >= 128*j + k_local
            masks = const.tile([P, 4, SC], BF16)
            for j in range(4):
                nc.gpsimd.memset(f32tmp, 1.0)
                nc.gpsimd.affine_select(
                    f32tmp, f32tmp, pattern=[[1, SC]],
                    compare_op=mybir.AluOpType.is_ge,
                    fill=0.0, base=-P * j, channel_multiplier=-1,
                )
                nc.vector.tensor_copy(masks[:, j, :], f32tmp)

            bias_keps = const.tile([P, 1], F32)
            nc.gpsimd.memset(bias_keps, float(P) * EPS)
            bias_qeps = const.tile([P, 1], F32)
            nc.gpsimd.memset(bias_qeps, EPS)

            # ---- persistent tensors ----
            KR = const.tile([P, S], BF16)        # roped+normed K
            Vs = const.tile([P, NKT, P], BF16)   # V, [s-in-tile, k-tile, d]
            qr_all = const.tile([P, G, S], BF16)  # roped+normed Q per head

            # tables + wo arrive during chunk-0/1 projections
            ck_sb = const.tile([P, S], BF16)
            sk_sb = const.tile([P, S], BF16)
            cq_sb = const.tile([P, S], BF16)
            sq_sb = const.tile([P, S], BF16)
            wo_sb = const.tile([P, G, HID], BF16)

            def emit_table_loads():
                nc.sync.dma_start(ck_sb, cosk)
                nc.sync.dma_start(sk_sb, sink)
                nc.sync.dma_start(cq_sb, cosq)
                nc.sync.dma_start(sq_sb, sinq)

            def emit_wo_load():
                for mt in range(G):
                    nc.sync.dma_start(wo_sb[:, mt, :],
                                      woT[mt * P:(mt + 1) * P, :])

            # ================= PHASE A: projections + rope/norm ==========
            acc_tiles = {}
            raw_tiles = {}

            def emit_proj(sc, pre_x=(), mid_pe_hook=None):
                q0 = sc * SC
                kvps = ps.tile([P, 2, SC], F32, tag="dual", bufs=3,
                               name=f"kvps{sc}")
                qps = [ps.tile([P, 2, SC], F32, tag="dual", bufs=3,
                               name=f"qps{sc}_{i}") for i in range(2)]
                xt = None
                for ht in range(NHT):
                    if ht == 6 and mid_pe_hook is not None:
                        mid_pe_hook()
                    if ht % XB == 0:
                        bi = ht // XB
                        if bi < len(pre_x):
                            xt = pre_x[bi]
                        else:
                            xt = xw.tile([P, XB, SC], BF16, tag="xt",
                                         name=f"xt{sc}_{ht}")
                            nc.sync.dma_start(
                                xt, xT[ht * P:(ht + XB) * P, q0:q0 + SC]
                                .rearrange("(b p) c -> p b c", p=P))
                    st = ht == 0
                    sp = ht == NHT - 1
                    xs = xt[:, ht % XB, :]
                    for mt in range(G):
                        nc.tensor.matmul(
                            qps[mt // 2][:, mt % 2, :],
                            wq_sb[:, ht, mt * P:(mt + 1) * P], xs,
                            start=st, stop=sp,
                        )
                    nc.tensor.matmul(kvps[:, 0, :], wk_sb[:, ht, :], xs,
                                     start=st, stop=sp)
                    nc.tensor.matmul(kvps[:, 1, :], wv_sb[:, ht, :], xs,
                                     start=st, stop=sp)
                acc_tiles[sc] = (kvps, qps)

            def emit_casts(sc):
                # PSUM -> SBUF casts only: frees the dual accumulator slots
                # in the order the next chunk's projections re-allocate them.
                kvps, qps = acc_tiles.pop(sc)
                kraw = scr.tile([P, SC], BF16, tag="kraw", name=f"kraw{sc}")
                nc.vector.tensor_copy(kraw, kvps[:, 0, :])
                vtmp = scr.tile([P, SC], BF16, tag="vtmp", name=f"vtmp{sc}")
                nc.vector.tensor_copy(vtmp, kvps[:, 1, :])
                qraw = scr.tile([P, G, SC], BF16, tag="qraw", bufs=1,
                                name=f"qraw{sc}")
                for h in range(G):
                    nc.vector.tensor_copy(qraw[:, h, :],
                                          qps[h // 2][:, h % 2, :])
                raw_tiles[sc] = (kraw, vtmp, qraw)

            norm_tiles = {}

            def emit_norms(sc):
                # rms-norm sums + V transposes: PE work that must NOT sit
                # behind a full projection chunk in the PE stream (the
                # blocked DVE consumers would stall the next chunk's casts).
                # Emitted mid-way through the NEXT chunk's projection MMs.
                kraw, vtmp, qraw = raw_tiles[sc]
                sqk = scr.tile([P, SC], BF16, tag="sq2", name=f"sqk{sc}")
                nc.vector.tensor_mul(sqk, kraw, kraw)
                ssb = ps.tile([P, SC], F32, tag="single", bufs=2,
                              name=f"ssbk{sc}")
                nc.tensor.matmul(ssb, ones_bf, sqk, start=True, stop=True)
                sqs = scr.tile([P, SC], F32, tag="sqs", bufs=1,
                               name=f"sqsk{sc}")
                nc.scalar.activation(sqs, ssb, Sqrt, bias=bias_keps,
                                     scale=1.0)
                rkf = scr.tile([P, SC], F32, tag="rqf", name=f"rkf{sc}")
                nc.vector.reciprocal_approx_fast(rkf, sqs)
                krot = scr.tile([P, SC], BF16, tag="krot", name=f"krot{sc}")
                nc.sync.dma_start(krot[0:64], kraw[64:128])
                nc.sync.dma_start(krot[64:128], kraw[0:64])
                qrot = scr.tile([P, G, SC], BF16, tag="qrot", bufs=1,
                                name=f"qrot{sc}")
                nc.sync.dma_start(qrot[0:64], qraw[64:128])
                nc.sync.dma_start(qrot[64:128], qraw[0:64])
                # V transposes
                for j in range(SC // P):
                    tp = ps.tile([P, P], BF16, tag="single", bufs=2,
                                 name=f"tp{sc}_{j}")
                    nc.tensor.transpose(tp, vtmp[:, j * P:(j + 1) * P],
                                        identity)
                    nc.vector.tensor_copy(Vs[:, sc * 4 + j, :], tp)
                rqfs = []
                for h in range(G):
                    sqq = scr.tile([P, SC], BF16, tag="sq2",
                                   name=f"sqq{sc}_{h}")
                    nc.vector.tensor_mul(sqq, qraw[:, h, :], qraw[:, h, :])
                    ssbq = ps.tile([P, SC], F32, tag="single", bufs=2,
                                   name=f"ssbq{sc}_{h}")
                    nc.tensor.matmul(ssbq, ones_bf, sqq, start=True,
                                     stop=True)
                    sqs_q = scr.tile([P, SC], F32, tag="sqs", bufs=1,
                                     name=f"sqsq{sc}_{h}")
                    nc.scalar.activation(sqs_q, ssbq, Sqrt, bias=bias_qeps,
                                         scale=1.0 / P)
                    rqf = scr.tile([P, SC], F32, tag="rqf",
                                   name=f"rqf{sc}_{h}")
                    nc.vector.reciprocal_approx_fast(rqf, sqs_q)
                    rqfs.append(rqf)
                norm_tiles[sc] = (rkf, krot, qrot, rqfs)

            def emit_rope_tail(sc):
                q0 = sc * SC
                kraw, vtmp, qraw = raw_tiles.pop(sc)
                rkf, krot, qrot, rqfs = norm_tiles.pop(sc)
                kt1 = scr.tile([P, SC], BF16, tag="t1", name=f"kt1{sc}")
                nc.vector.tensor_mul(kt1, krot, sk_sb[:, q0:q0 + SC])
                kpre = scr.tile([P, SC], BF16, tag="pre", name=f"kpre{sc}")
                nc.vector.tensor_mul(kpre, kraw, ck_sb[:, q0:q0 + SC])
                nc.vector.tensor_add(kpre, kpre, kt1)
                nc.vector.tensor_mul(KR[:, q0:q0 + SC], kpre, rkf)
                for h in range(G):
                    t1 = scr.tile([P, SC], BF16, tag="t1",
                                  name=f"qt1{sc}_{h}")
                    nc.vector.tensor_mul(t1, qrot[:, h, :],
                                         sq_sb[:, q0:q0 + SC])
                    qpre = scr.tile([P, SC], BF16, tag="pre",
                                    name=f"qpre{sc}_{h}")
                    nc.vector.tensor_mul(qpre, qraw[:, h, :],
                                         cq_sb[:, q0:q0 + SC])
                    nc.vector.tensor_add(qpre, qpre, t1)
                    nc.vector.tensor_mul(qr_all[:, h, q0:q0 + SC], qpre,
                                         rqfs[h])

            # ================= PHASE B emitter ===========================
            def emit_attn_chunk(sc):
                q0 = sc * SC
                ots = [None] * G
                pending = None  # delayed dn/rcp/ot tail of the previous head

                def make_tail(h, avp, acc):
                    def tail():
                        # denominator partition-reduce: ONE ones-matmul on
                        # the DVE-accumulated exp sums instead of one
                        # matmul per k-tile. Emitted a head late so the
                        # DVE accumulation chain never stalls the PE.
                        dnp = ps.tile([P, SC], F32, tag="dual", bufs=3,
                                      name=f"dnp{sc}_{h}")
                        nc.tensor.matmul(dnp, ones_r, acc,
                                         start=True, stop=True)
                        rcp = scr.tile([P, SC], F32, tag="rcp",
                                       name=f"rcp{sc}_{h}")
                        nc.vector.reciprocal_approx_fast(rcp, dnp)
                        ot = otp.tile([P, SC], BF16, tag="ot",
                                      name=f"ot{sc}_{h}")
                        nc.vector.tensor_mul(ot, avp, rcp)
                        ots[h] = ot
                    return tail

                for h in range(G):
                    avp = ps.tile([P, SC], F32, tag="single", bufs=2,
                                  name=f"avp{sc}_{h}")
                    acc = scr.tile([P, SC], mybir.dt.float32r, tag="acc",
                                   name=f"acc{sc}_{h}")
                    nkt = (sc + 1) * 4
                    for kp in range(nkt // 2):
                        ptps = ps.tile([P, 2, SC], F32, tag="dual", bufs=3,
                                       name=f"ptps{sc}_{h}_{kp}")
                        for j in range(2):
                            kt = 2 * kp + j
                            nc.tensor.matmul(
                                ptps[:, j, :], KR[:, kt * P:(kt + 1) * P],
                                qr_all[:, h, q0:q0 + SC],
                                start=True, stop=True,
                            )
                        pt = ptp.tile([P, 2, SC], BF16, tag="pt",
                                      name=f"pt{sc}_{h}_{kp}")
                        nc.scalar.activation(pt, ptps, Exp)
                        if 2 * kp >= sc * 4:
                            jm = 2 * kp - sc * 4
                            nc.vector.tensor_mul(pt, pt,
                                                 masks[:, jm:jm + 2, :])
                        for j in range(2):
                            kt = 2 * kp + j
                            nc.tensor.matmul(
                                avp, Vs[:, kt, :], pt[:, j, :],
                                start=(kt == 0), stop=(kt == nkt - 1))
                        psb = scr.tile([P, SC], BF16, tag="psb",
                                       name=f"psb{sc}_{h}_{kp}")
                        nc.vector.tensor_add(psb, pt[:, 0, :], pt[:, 1, :])
                        if kp == 0:
                            nc.vector.tensor_copy(acc, psb)
                        else:
                            nc.vector.tensor_add(acc, acc, psb)
                        if kp == 0 and pending is not None:
                            pending()
                            pending = None
                    pending = make_tail(h, avp, acc)
                pending()

                # ---- output projection for this q chunk ----
                for stl in range(SC // P):
                    srow = q0 + stl * P
                    ys = ysp.tile([P, HID], BF16, tag="ys",
                                  name=f"ys{sc}_{stl}")
                    for hc in range(HID // SC):
                        yps = ps.tile([P, SC], F32, tag="single", bufs=2,
                                      name=f"yps{sc}_{stl}_{hc}")
                        for h in range(G):
                            nc.tensor.matmul(
                                yps, ots[h][:, stl * P:(stl + 1) * P],
                                wo_sb[:, h, hc * SC:(hc + 1) * SC],
                                start=(h == 0), stop=(h == G - 1),
                            )
                        nc.vector.tensor_copy(
                            ys[:, hc * SC:(hc + 1) * SC], yps)
                    nc.sync.dma_start(y[srow:srow + P, :], ys)

            # ================= program order ==============================
            emit_proj(0, pre_x=x0_tiles)
            emit_table_loads()
            for sc in range(1, NSC):
                emit_casts(sc - 1)
                emit_proj(sc, mid_pe_hook=lambda s=sc - 1: emit_norms(s))
                if sc == 1:
                    emit_wo_load()
                emit_rope_tail(sc - 1)
            emit_casts(NSC - 1)
            emit_norms(NSC - 1)
            emit_attn_chunk(0)
            emit_rope_tail(NSC - 1)
            for sc in range(1, NSC):
                emit_attn_chunk(sc)

    nc.finalize()
    return nc


def shard_inputs(x, wq, wk, wv, wo, q_norm_w, k_norm_w, cos_table, sin_table,
                 positions, **_ignored):
    """Host-side sharding: returns the list of 8 per-core input maps."""
    import ml_dtypes
    bf16 = ml_dtypes.bfloat16

    x = np.asarray(x, np.float32)
    pos = np.asarray(positions).astype(np.int64)
    cos_sel = np.asarray(cos_table, np.float32)[pos]   # [S, D]
    sin_sel = np.asarray(sin_table, np.float32)[pos]
    qw = np.asarray(q_norm_w, np.float32)
    kw = np.asarray(k_norm_w, np.float32)
    # fold norm weights into the transposed rope tables:
    # w * rope(q') == q'*(w*cos) + rot(q')*(w*sin)
    # also fold rotate-half's minus sign into sin rows 0..63:
    # rope(z) = z*cos + [-z2; z1]*sin = z*cos + [z2; z1]*sin_eff
    sign = np.ones((1, D), np.float32)
    sign[0, :D // 2] = -1.0
    cosq = np.ascontiguousarray((cos_sel * qw).T).astype(bf16)     # [D, S]
    sinq = np.ascontiguousarray((sin_sel * qw * sign).T).astype(bf16)
    cosk = np.ascontiguousarray((cos_sel * kw).T).astype(bf16)
    sink = np.ascontiguousarray((sin_sel * kw * sign).T).astype(bf16)
    xTf = np.ascontiguousarray(x.reshape(S, HID).T).astype(bf16)   # [HID, S]
    wq = np.asarray(wq, np.float32)
    wk = np.asarray(wk, np.float32)
    wv = np.asarray(wv, np.float32)
    wo = np.asarray(wo, np.float32)

    in_maps = []
    for c in range(N_CORES):
        m = {
            "xT": xTf,
            "wqT": np.ascontiguousarray(
                wq[c * G * P:(c + 1) * G * P, :].T).astype(bf16),
            "wkT": np.ascontiguousarray(
                wk[c * P:(c + 1) * P, :].T).astype(bf16),
            "wvT": np.ascontiguousarray(
                wv[c * P:(c + 1) * P, :].T).astype(bf16),
            "woT": np.ascontiguousarray(
                wo[:, c * G * P:(c + 1) * G * P].T).astype(bf16),
            "cosq": cosq, "sinq": sinq, "cosk": cosk, "sink": sink,
        }
        in_maps.append(m)
    return in_maps


_NC = None


def _get_nc():
    global _NC
    if _NC is None:
        _NC = build_program()
    return _NC


def run_on_device(in_maps, trace=False, **kw):
    from concourse.bass_utils import run_bass_kernel_spmd
    nc = _get_nc()
    return run_bass_kernel_spmd(nc, in_maps, list(range(N_CORES)),
                                trace=trace, **kw)


def kernel(**inputs):
    in_maps = shard_inputs(**inputs)
    res = run_on_device(in_maps).results
    y = np.zeros((S, HID), np.float32)
    for c in range(N_CORES):
        y += np.asarray(res[c]["y"], np.float32)
    return y.reshape(1, S, HID)
